# revision 1
# baseline (speedup 1.0000x reference)
"""AttnBlock (GroupNorm + 1x1-conv QKV + spatial attention + proj + residual)
as a Bass/Tile kernel for 8 Trainium2 NeuronCores.

Sharding: data-parallel over the folded B*T=16 frame axis -> 2 frames/core.
Params replicated. Each core runs an identical program on its own frame pair.

Layout conventions (per frame):
  x, h, q, k, att, out : SBUF [128, KO=4, HW=1024], channel c = ko*128 + p
  vT                   : SBUF [128, SO=8, C=512],   spatial s = so*128 + p
  E = exp(scale*S^T)   : SBUF [128, SO=8, 512],     j = jo*128 + p, i chunk of 512

All matmuls run as float32r (fp32 bits, reduced-precision full-rate PE mode).
Softmax is computed without max-subtraction (scores are O(1) for this
problem), with the row-sum Z obtained by an extra ones-matmul over E and the
division folded into the PSUM->SBUF drain of the A@V result.
"""

from contextlib import ExitStack

import numpy as np

import concourse.bass as bass
import concourse.bacc as bacc
import concourse.mybir as mybir
import concourse.tile as tile
from concourse.bass import ts
from concourse.bass_utils import run_bass_kernel_spmd

# Problem shapes (hardcoded per harness contract)
B, T, C, H, W = 2, 8, 512, 32, 32
HW = H * W              # 1024
FRAMES = B * T          # 16
NCORES = 8
FPC = FRAMES // NCORES  # frames per core
P = 128
KO = C // P             # 4 channel blocks
SO = HW // P            # 8 spatial blocks
NCH = HW // 512         # 2 free chunks of 512
EPS = 1e-6
SCALE = float(C) ** -0.5

F32 = mybir.dt.float32
F32R = mybir.dt.float32r
AF = mybir.ActivationFunctionType
OP = mybir.AluOpType




def _build(reps=1):
    nc = bacc.Bacc(None, target_bir_lowering=False)
    d = {}
    d["x"] = nc.dram_tensor("x", [FPC, P, KO, HW], F32, kind="ExternalInput")
    for nm in ("wq", "wk", "wv", "wp"):
        d[nm] = nc.dram_tensor(nm, [P, KO, C], F32R, kind="ExternalInput")
    for nm in ("bq", "bk", "bp", "gns", "gnb"):
        d[nm] = nc.dram_tensor(nm, [P, KO], F32, kind="ExternalInput")
    d["bvb"] = nc.dram_tensor("bvb", [P, C], F32, kind="ExternalInput")
    d["ones"] = nc.dram_tensor("ones", [P, P], F32R, kind="ExternalInput")
    d["out"] = nc.dram_tensor("out", [FPC, P, KO, HW], F32, kind="ExternalOutput")

    # Constant matrices for the group-stat partition reductions (baked in NEFF)
    aggA = np.zeros((P, 8), np.float32)
    for pp in range(P):
        aggA[pp, pp // 16] = 1.0 / 16.0
    expB = np.zeros((8, P), np.float32)
    for pp in range(P):
        expB[pp // 16, pp] = 1.0
    d["aggA"] = nc.inline_tensor(aggA, "aggA")
    d["expB"] = nc.inline_tensor(expB, "expB")

    with tile.TileContext(nc) as tc:
        with ExitStack() as ctx:
            _emit(ctx, nc, tc, d, reps)
    nc.compile()
    return nc


def _emit(ctx, nc, tc, d, reps=1):
    const = ctx.enter_context(tc.tile_pool(name="const", bufs=1))
    px = ctx.enter_context(tc.tile_pool(name="px", bufs=2))
    phf = ctx.enter_context(tc.tile_pool(name="phf", bufs=2))
    pq = ctx.enter_context(tc.tile_pool(name="pqp", bufs=1))
    pk = ctx.enter_context(tc.tile_pool(name="pkp", bufs=2))
    pv = ctx.enter_context(tc.tile_pool(name="pvp", bufs=1))
    pe_ = ctx.enter_context(tc.tile_pool(name="pep", bufs=1))
    prz = ctx.enter_context(tc.tile_pool(name="przp", bufs=2))
    pgn = ctx.enter_context(tc.tile_pool(name="pgn", bufs=2))
    psum = ctx.enter_context(tc.tile_pool(name="psum", bufs=5, space="PSUM"))
    psg = ctx.enter_context(tc.tile_pool(name="psg", bufs=1, space="PSUM"))
    psg2 = ctx.enter_context(tc.tile_pool(name="psg2", bufs=2, space="PSUM"))

    # ---- load replicated params ----
    # frame 0/1 input DMAs go FIRST on the SP HWDGE ring (FIFO per ring);
    # params ride the ACT ring so they don't delay the first frame.
    import math as _math

    xfs = {}
    for f0 in range(min(2, FPC * reps)):
        xf = px.tile([P, KO, HW], F32, tag="xf", name=f"xf{f0}")
        nc.sync.dma_start(out=xf[:], in_=d["x"].ap()[f0 % FPC])
        xfs[f0] = xf

    # warm the ACT table set (Exp/Ln/Identity) at t~0: eps8 = exp(ln(eps))
    epsl = const.tile([8, 1], F32, tag="epsl", name="epsl")
    eps8 = const.tile([8, 1], F32, tag="eps8", name="eps8")
    nc.vector.memset(epsl[:], _math.log(EPS))
    nc.scalar.activation(out=eps8[:], in_=epsl[:], func=AF.Exp)

    aggA_s = const.tile([P, 8], F32, tag="aggA_s", name="aggA_s")
    nc.scalar.dma_start(out=aggA_s[:], in_=d["aggA"].ap())
    expB_s = const.tile([8, P], F32, tag="expB_s", name="expB_s")
    nc.scalar.dma_start(out=expB_s[:], in_=d["expB"].ap())
    ws = {}
    for nm in ("wq", "wk", "wv", "wp"):
        t = const.tile([P, KO, C], F32R, tag=f"{nm}_s", name=f"{nm}_s")
        nc.scalar.dma_start(out=t[:], in_=d[nm].ap())
        ws[nm] = t
    ones_s = const.tile([P, P], F32R, tag="ones_s", name="ones_s")
    nc.scalar.dma_start(out=ones_s[:], in_=d["ones"].ap())
    small = {}
    for nm in ("bq", "bk", "bp", "gns", "gnb"):
        t = const.tile([P, KO], F32, tag=f"{nm}_s", name=f"{nm}_s")
        nc.scalar.dma_start(out=t[:], in_=d[nm].ap())
        small[nm] = t
    bvb_s = const.tile([P, C], F32, tag="bvb_s", name="bvb_s")
    nc.scalar.dma_start(out=bvb_s[:], in_=d["bvb"].ap())

    hfs, qs, ks, vts, atts = {}, {}, {}, {}, {}

    def load_gn(f):
        """DMA frame f in; GroupNorm stats + normalize (fused affine)."""
        if f in xfs:
            xf = xfs[f]
        else:
            xf = px.tile([P, KO, HW], F32, tag="xf", name=f"xf{f}")
            nc.sync.dma_start(out=xf[:], in_=d["x"].ap()[f % FPC])
        # per-channel mean/var over the 1024 spatial positions
        stats = pgn.tile([P, KO, 2, 6], F32, tag="stats", name=f"stats{f}")
        for ko in range(KO):
            for hh in range(2):
                nc.vector.bn_stats(out=stats[:, ko, hh, :], in_=xf[:, ko, ts(hh, 512)])
        mv = pgn.tile([P, KO, 2], F32, tag="mv", name=f"mv{f}")
        for ko in range(KO):
            nc.vector.bn_aggr(out=mv[:, ko, :], in_=stats[:, ko, :, :])
        # columns: (mean_c, var_c + mean_c^2) = (mean_c, E[x^2]_c)
        agg_in = pgn.tile([P, KO, 2], F32, tag="agg_in", name=f"agg{f}")
        nc.vector.tensor_copy(out=agg_in[:, :, 0], in_=mv[:, :, 0])
        nc.vector.tensor_tensor(
            out=agg_in[:, :, 1], in0=mv[:, :, 0], in1=mv[:, :, 0], op=OP.mult
        )
        nc.vector.tensor_tensor(
            out=agg_in[:, :, 1], in0=agg_in[:, :, 1], in1=mv[:, :, 1], op=OP.add
        )
        # group-aggregate 16 channels (partitions) per group: [8, (ko,stat)]
        gps = psg.tile([8, 8], F32, tag="gps", name=f"gps{f}")
        if f == 0:
            # warm-up: absorb each const/weight DMA wait into its own dummy
            # matmul so every real matmul carries at most one sync wait
            # (the fused-LDW matmul instruction has a single wait slot).
            warm_slices = [ws[wnm][:, 0, :8] for wnm in ("wq", "wk", "wv", "wp")]
            warm_slices.append(ones_s[:, :8])
            for wsl in warm_slices:
                nc.tensor.matmul(
                    gps[:], lhsT=wsl, rhs=wsl, start=True, stop=True,
                )
            nc.tensor.matmul(
                gps[:], lhsT=aggA_s[:], rhs=aggA_s[:], start=True, stop=True,
            )
        nc.tensor.matmul(
            gps[:],
            lhsT=aggA_s[:],
            rhs=agg_in[:].rearrange("p a b -> p (a b)"),
            start=True,
            stop=True,
        )
        gpsv = gps[:].rearrange("p (a b) -> p a b", b=2)
        # var_g = E[x^2]_g - mean_g^2 (DVE may read only one PSUM operand)
        mean8 = pgn.tile([8, KO], F32, tag="mean8", name=f"mean8{f}")
        tmp8 = pgn.tile([8, KO], F32, tag="tmp8", name=f"tmp8{f}")
        vv = pgn.tile([8, KO], F32, tag="vv", name=f"vv{f}")
        nc.vector.tensor_copy(out=mean8[:], in_=gpsv[:, :, 0])
        nc.vector.tensor_tensor(
            out=tmp8[:], in0=mean8[:], in1=mean8[:], op=OP.mult
        )
        nc.vector.tensor_tensor(
            out=vv[:], in0=gpsv[:, :, 1], in1=tmp8[:], op=OP.subtract
        )
        # gs2 written only by ACT so the expand matmul needs a single wait:
        # col0 = mean (copy from PSUM), col1 = rstd = exp(-0.5*ln(var+eps))
        gs2 = pgn.tile([8, KO, 2], F32, tag="gs2", name=f"gs2{f}")
        nc.scalar.activation(out=gs2[:, :, 0], in_=gpsv[:, :, 0], func=AF.Copy)
        nc.scalar.activation(out=gs2[:, :, 1], in_=vv[:], func=AF.Ln, bias=eps8[:])
        nc.scalar.activation(
            out=gs2[:, :, 1], in_=gs2[:, :, 1], func=AF.Exp, scale=-0.5
        )
        # broadcast group stats back to the 128 channel partitions
        gpe = psg2.tile([P, 8], F32, tag="gpe", name=f"gpe{f}")
        if f == 0:
            nc.tensor.matmul(
                gpe[:8], lhsT=expB_s[:, :8], rhs=expB_s[:, :8],
                start=True, stop=True,
            )
        nc.tensor.matmul(
            gpe[:],
            lhsT=expB_s[:],
            rhs=gs2[:].rearrange("p a b -> p (a b)"),
            start=True,
            stop=True,
        )
        gpev = gpe[:].rearrange("p (a b) -> p a b", b=2)
        # fold GN affine: h = x*(rstd*s) + (b - mean*rstd*s)
        scp = pgn.tile([P, KO], F32, tag="scp", name=f"scp{f}")
        bip = pgn.tile([P, KO], F32, tag="bip", name=f"bip{f}")
        tmpp = pgn.tile([P, KO], F32, tag="tmpp", name=f"tmpp{f}")
        nc.vector.tensor_tensor(
            out=scp[:], in0=gpev[:, :, 1], in1=small["gns"][:], op=OP.mult
        )
        nc.vector.tensor_tensor(
            out=tmpp[:], in0=gpev[:, :, 0], in1=scp[:], op=OP.mult
        )
        nc.vector.tensor_tensor(
            out=bip[:], in0=small["gnb"][:], in1=tmpp[:], op=OP.subtract
        )
        hf = phf.tile([P, KO, HW], F32R, tag="hfout", name=f"hf{f}")
        for ko in range(KO):
            nc.vector.tensor_scalar(
                out=hf[:, ko, :],
                in0=xf[:, ko, :],
                scalar1=scp[:, ko : ko + 1],
                scalar2=bip[:, ko : ko + 1],
                op0=OP.mult,
                op1=OP.add,
            )
        xfs[f], hfs[f] = xf, hf

    def qkv(f):
        hf = hfs[f]
        qf = pq.tile([P, KO, HW], F32R, tag="qf", name=f"qf{f}")
        kf = pk.tile([P, KO, HW], F32R, tag="katt", name=f"kf{f}")
        for wt, bt, dst in ((ws["wq"], small["bq"], qf), (ws["wk"], small["bk"], kf)):
            for mi in range(KO):
                for ic in range(NCH):
                    pt = psum.tile([P, 512], F32, tag="pb", name="pt")
                    for ki in range(KO):
                        nc.tensor.matmul(
                            pt[:],
                            lhsT=wt[:, ki, ts(mi, P)],
                            rhs=hf[:, ki, ts(ic, 512)],
                            start=(ki == 0),
                            stop=(ki == KO - 1),
                        )
                    nc.scalar.activation(
                        out=dst[:, mi, ts(ic, 512)],
                        in_=pt[:],
                        func=AF.Identity,
                        bias=bt[:, mi : mi + 1],
                    )
        vt = pv.tile([P, SO, C], F32R, tag="vt", name=f"vt{f}")
        for so in range(SO):
            pt = psum.tile([P, 512], F32, tag="pb", name="pt")
            for ki in range(KO):
                nc.tensor.matmul(
                    pt[:],
                    lhsT=hf[:, ki, ts(so, P)],
                    rhs=ws["wv"][:, ki, :],
                    start=(ki == 0),
                    stop=(ki == KO - 1),
                )
            nc.vector.tensor_add(out=vt[:, so, :], in0=pt[:], in1=bvb_s[:])
        qs[f], ks[f], vts[f] = qf, kf, vt

    def attn(f):
        qf, kf, vt = qs[f], ks[f], vts[f]
        att = pk.tile([P, KO, HW], F32R, tag="katt", name=f"att{f}")
        for ic in range(NCH):
            # E = exp(scale * S^T) for this i-chunk, j on partitions
            ef = pe_.tile([P, SO, 512], F32R, tag="ef", name=f"ef{f}_{ic}")
            for jo in range(SO):
                pt = psum.tile([P, 512], F32, tag="pb", name="pt")
                for ki in range(KO):
                    nc.tensor.matmul(
                        pt[:],
                        lhsT=kf[:, ki, ts(jo, P)],
                        rhs=qf[:, ki, ts(ic, 512)],
                        start=(ki == 0),
                        stop=(ki == KO - 1),
                    )
                nc.scalar.activation(out=ef[:, jo, :], in_=pt[:], func=AF.Exp, scale=SCALE)
            # Z_i = sum_j E[j,i], broadcast to all partitions via ones-matmul
            pz = psum.tile([P, 512], F32, tag="pb", name="pt")
            for jo in range(SO):
                nc.tensor.matmul(
                    pz[:],
                    lhsT=ones_s[:],
                    rhs=ef[:, jo, :],
                    start=(jo == 0),
                    stop=(jo == SO - 1),
                )
            rz = prz.tile([P, 512], F32, tag="rz", name=f"rz{f}_{ic}")
            nc.vector.reciprocal(out=rz[:], in_=pz[:])
            # att[c, i] = (sum_j vT[j,c] E[j,i]) / Z_i
            for mi in range(KO):
                pt = psum.tile([P, 512], F32, tag="pb", name="pt")
                for jo in range(SO):
                    nc.tensor.matmul(
                        pt[:],
                        lhsT=vt[:, jo, ts(mi, P)],
                        rhs=ef[:, jo, :],
                        start=(jo == 0),
                        stop=(jo == SO - 1),
                    )
                nc.vector.tensor_mul(
                    out=att[:, mi, ts(ic, 512)], in0=pt[:], in1=rz[:]
                )
        atts[f] = att

    def proj(f):
        att, xf = atts[f], xfs[f]
        of = phf.tile([P, KO, HW], F32, tag="hfout", name=f"of{f}")
        for mi in range(KO):
            for ic in range(NCH):
                pt = psum.tile([P, 512], F32, tag="pb", name="pt")
                for ki in range(KO):
                    nc.tensor.matmul(
                        pt[:],
                        lhsT=ws["wp"][:, ki, ts(mi, P)],
                        rhs=att[:, ki, ts(ic, 512)],
                        start=(ki == 0),
                        stop=(ki == KO - 1),
                    )
                nc.scalar.activation(
                    out=of[:, mi, ts(ic, 512)],
                    in_=pt[:],
                    func=AF.Identity,
                    bias=small["bp"][:, mi : mi + 1],
                )
                nc.vector.tensor_add(
                    out=of[:, mi, ts(ic, 512)],
                    in0=of[:, mi, ts(ic, 512)],
                    in1=xf[:, mi, ts(ic, 512)],
                )
            nc.sync.dma_start(
                out=d["out"].ap()[f % FPC, :, mi], in_=of[:, mi]
            )

    # Emission order = scheduling priority. Hoist frame f+1's load+GN ahead of
    # frame f's attention so the frame-boundary normalize overlaps PE work.
    nvf = FPC * reps
    load_gn(0)
    qkv(0)
    if nvf > 1:
        load_gn(1)
    for f in range(nvf):
        attn(f)
        proj(f)
        if f + 1 < nvf:
            qkv(f + 1)
        if f + 2 < nvf:
            load_gn(f + 2)


_NC_CACHE = None


def _get_nc():
    global _NC_CACHE
    if _NC_CACHE is None:
        _NC_CACHE = _build()
    return _NC_CACHE


def _wprep(w):
    # w [Cout, Cin] -> lhsT layout [P, KO(ki), Cout], cin = ki*128 + p
    w = np.asarray(w, np.float32)
    return np.ascontiguousarray(w.T.reshape(KO, P, C).transpose(1, 0, 2))


def _bprep(b):
    # b [C] -> [P, KO], c = ko*128 + p
    return np.ascontiguousarray(np.asarray(b, np.float32).reshape(KO, P).T)


def _prep(inputs):
    x = np.asarray(inputs["x"], dtype=np.float32)
    base = {
        "wq": _wprep(inputs["wq"]),
        "wk": _wprep(inputs["wk"]),
        "wv": _wprep(inputs["wv"]),
        "wp": _wprep(inputs["wproj"]),
        "bq": _bprep(inputs["bq"]),
        "bk": _bprep(inputs["bk"]),
        "bp": _bprep(inputs["bproj"]),
        "gns": _bprep(inputs["gn_scale"]),
        "gnb": _bprep(inputs["gn_bias"]),
        "ones": np.ones((P, P), np.float32),
        "bvb": np.ascontiguousarray(
            np.broadcast_to(np.asarray(inputs["bv"], np.float32), (P, C))
        ),
    }
    xs = x.reshape(FRAMES, KO, P, HW).transpose(0, 2, 1, 3)  # [16, P, KO, HW]
    in_maps = []
    for i in range(NCORES):
        m = dict(base)
        m["x"] = np.ascontiguousarray(xs[i * FPC : (i + 1) * FPC])
        in_maps.append(m)
    return in_maps


def _run(inputs, trace=False):
    nc = _get_nc()
    in_maps = _prep(inputs)
    res = run_bass_kernel_spmd(
        nc, in_maps, core_ids=list(range(NCORES)), trace=trace
    )
    outs = []
    for rmap in res.results:
        o = np.asarray(rmap["out"])  # [FPC, P, KO, HW]
        outs.append(o.transpose(0, 2, 1, 3).reshape(FPC, C, H, W))
    full = np.concatenate(outs, axis=0).reshape(B, T, C, H, W).astype(np.float32)
    return full, res


def kernel(**inputs):
    out, _ = _run(inputs, trace=False)
    return out



# revision 4
# speedup vs baseline: 1.3969x; 1.3969x over previous
"""AttnBlock (GroupNorm + 1x1-conv QKV + spatial attention + proj + residual)
as a Bass/Tile kernel for 8 Trainium2 NeuronCores.

Sharding: data-parallel over the folded B*T=16 frame axis -> 2 frames/core.
Params replicated. Each core runs an identical program on its own frame pair.

All heavy matmuls run in fp8e4 (e4m3) DoubleRow mode: the PE array is
virtualized to 128x256 (2 fp8 weights per cell), halving streaming cycles.
Weights are host-prescaled by 64 to keep them out of the fp8 subnormal range;
the scale is removed in the drains (q/k: *1/64, proj: *1/2048). V keeps the
64x (drain adds 64*bv), which cancels in softmax normalization because the
row-sum Z is accumulated with a ones=2.0 matrix: att = (64*AV)*(1/(2Z)) =
32*att_true, and the proj drain divides by 64*32 = 2048.

Layout conventions (per frame):
  x                    : SBUF fp32 [128, KO=4, HW=1024], channel c = ko*128+p
  h, q, k, att         : SBUF fp8  [128, KO=4, HW=1024]
  vT                   : SBUF fp8  [128, SO=8, C=512],   spatial s = so*128+p
  E = exp(scale*S^T)   : SBUF fp8  [128, SO=8, 512],     j = jo*128+p
Softmax has no max-subtraction (scores are O(1) for this problem).

Engine split: PE matmuls; ACT exp + proj drain; DVE q/k drains, v drain,
AV drain, bn_stats, reciprocal_approx_fast; GpSimd GN-normalize + residual.
"""

from contextlib import ExitStack

import numpy as np
import ml_dtypes

import concourse.bass as bass
import concourse.bacc as bacc
import concourse.mybir as mybir
import concourse.tile as tile
from concourse.bass import ts
from concourse.bass_utils import run_bass_kernel_spmd

# Problem shapes (hardcoded per harness contract)
B, T, C, H, W = 2, 8, 512, 32, 32
HW = H * W              # 1024
FRAMES = B * T          # 16
NCORES = 8
FPC = FRAMES // NCORES  # frames per core
P = 128
KO = C // P             # 4 channel blocks
SO = HW // P            # 8 spatial blocks
NCH = HW // 512         # 2 free chunks of 512
EPS = 1e-6
SCALE = float(C) ** -0.5
WS = 64.0               # host-side weight prescale (keeps fp8 out of denormals)

F32 = mybir.dt.float32
F8 = mybir.dt.float8e4
AF = mybir.ActivationFunctionType
OP = mybir.AluOpType
DR = mybir.MatmulPerfMode.DoubleRow


def _build(reps=1):
    nc = bacc.Bacc(None, target_bir_lowering=False)
    d = {}
    d["x"] = nc.dram_tensor("x", [FPC, P, KO, HW], F32, kind="ExternalInput")
    for nm in ("wq", "wk", "wv", "wp"):
        d[nm] = nc.dram_tensor(nm, [P, KO, C], F8, kind="ExternalInput")
    for nm in ("bq", "bk", "bp", "gns", "gnb"):
        d[nm] = nc.dram_tensor(nm, [P, KO], F32, kind="ExternalInput")
    d["bvb"] = nc.dram_tensor("bvb", [P, C], F32, kind="ExternalInput")
    d["ones"] = nc.dram_tensor("ones", [P, 2, P], F8, kind="ExternalInput")
    d["out"] = nc.dram_tensor("out", [FPC, P, KO, HW], F32, kind="ExternalOutput")

    # Constant matrices for the group-stat partition reductions (baked in NEFF)
    aggA = np.zeros((P, 8), np.float32)
    for pp in range(P):
        aggA[pp, pp // 16] = 1.0 / 16.0
    expB = np.zeros((8, P), np.float32)
    for pp in range(P):
        expB[pp // 16, pp] = 1.0
    d["aggA"] = nc.inline_tensor(aggA, "aggA")
    d["expB"] = nc.inline_tensor(expB, "expB")

    with tile.TileContext(nc) as tc:
        with ExitStack() as ctx:
            _emit(ctx, nc, tc, d, reps)
    nc.compile()
    return nc


def _emit(ctx, nc, tc, d, reps=1):
    const = ctx.enter_context(tc.tile_pool(name="const", bufs=1))
    px = ctx.enter_context(tc.tile_pool(name="px", bufs=3))
    phf = ctx.enter_context(tc.tile_pool(name="phf", bufs=2))
    pq = ctx.enter_context(tc.tile_pool(name="pqp", bufs=2))
    pk = ctx.enter_context(tc.tile_pool(name="pkp", bufs=2))
    pv = ctx.enter_context(tc.tile_pool(name="pvp", bufs=2))
    pe_ = ctx.enter_context(tc.tile_pool(name="pep", bufs=2))
    pat = ctx.enter_context(tc.tile_pool(name="patp", bufs=2))
    prz = ctx.enter_context(tc.tile_pool(name="przp", bufs=2))
    pgn = ctx.enter_context(tc.tile_pool(name="pgn", bufs=2))
    pof = ctx.enter_context(tc.tile_pool(name="pofp", bufs=2))
    psum = ctx.enter_context(tc.tile_pool(name="psum", bufs=5, space="PSUM"))
    psg = ctx.enter_context(tc.tile_pool(name="psg", bufs=1, space="PSUM"))
    psg2 = ctx.enter_context(tc.tile_pool(name="psg2", bufs=2, space="PSUM"))

    import math as _math

    # ---- DMA issue order matters: per-ring FIFO. ----
    # Frame-0 x slices first (sync ring, per-ko for early GN start); weights
    # split across the vector/gpsimd rings so no single queue serializes them
    # and the scalar ring's ACT-table loads don't delay them.
    xfs = {}
    for f0 in range(min(2, FPC * reps)):
        xf = px.tile([P, KO, HW], F32, tag="xf", name=f"xf{f0}")
        for ko in range(KO):
            nc.sync.dma_start(out=xf[:, ko], in_=d["x"].ap()[f0 % FPC, :, ko])
        xfs[f0] = xf

    ws = {}
    for nm, eng in (("wq", nc.scalar), ("wk", nc.scalar),
                    ("wv", nc.gpsimd), ("wp", nc.gpsimd)):
        t = const.tile([P, KO, C], F8, tag=f"{nm}_s", name=f"{nm}_s")
        eng.dma_start(out=t[:], in_=d[nm].ap())
        ws[nm] = t
    ones_s = const.tile([P, 2, P], F8, tag="ones_s", name="ones_s")
    nc.scalar.dma_start(out=ones_s[:], in_=d["ones"].ap())
    bvb_s = const.tile([P, C], F32, tag="bvb_s", name="bvb_s")
    nc.gpsimd.dma_start(out=bvb_s[:], in_=d["bvb"].ap())
    aggA_s = const.tile([P, 8], F32, tag="aggA_s", name="aggA_s")
    nc.scalar.dma_start(out=aggA_s[:], in_=d["aggA"].ap())
    expB_s = const.tile([8, P], F32, tag="expB_s", name="expB_s")
    nc.scalar.dma_start(out=expB_s[:], in_=d["expB"].ap())
    small = {}
    for nm in ("bq", "bk", "bp", "gns", "gnb"):
        t = const.tile([P, KO], F32, tag=f"{nm}_s", name=f"{nm}_s")
        nc.scalar.dma_start(out=t[:], in_=d[nm].ap())
        small[nm] = t

    # Warm the ACT table set (Exp/Ln/Identity/Copy) during the DMA window:
    # eps8 = exp(ln(eps)); tiny Identity+Copy on the result.
    epsl = const.tile([8, 1], F32, tag="epsl", name="epsl")
    eps8 = const.tile([8, 1], F32, tag="eps8", name="eps8")
    scr8 = const.tile([8, 1], F32, tag="scr8", name="scr8")
    nc.vector.memset(epsl[:], _math.log(EPS))
    nc.scalar.activation(out=eps8[:], in_=epsl[:], func=AF.Exp)
    nc.scalar.activation(out=scr8[:], in_=eps8[:], func=AF.Identity)
    nc.scalar.activation(out=scr8[:], in_=eps8[:], func=AF.Copy)

    hfs, qs, ks, vts, atts = {}, {}, {}, {}, {}

    def load_gn(f):
        """DMA frame f in; GroupNorm stats + normalize (fused affine)."""
        if f in xfs:
            xf = xfs[f]
        else:
            xf = px.tile([P, KO, HW], F32, tag="xf", name=f"xf{f}")
            for ko in range(KO):
                nc.sync.dma_start(out=xf[:, ko], in_=d["x"].ap()[f % FPC, :, ko])
        # per-channel mean/var over the 1024 spatial positions
        stats = pgn.tile([P, KO, 2, 6], F32, tag="stats", name=f"stats{f}")
        for ko in range(KO):
            for hh in range(2):
                nc.vector.bn_stats(out=stats[:, ko, hh, :], in_=xf[:, ko, ts(hh, 512)])
        mv = pgn.tile([P, KO, 2], F32, tag="mv", name=f"mv{f}")
        for ko in range(KO):
            nc.vector.bn_aggr(out=mv[:, ko, :], in_=stats[:, ko, :, :])
        # columns: (mean_c, var_c + mean_c^2) = (mean_c, E[x^2]_c)
        agg_in = pgn.tile([P, KO, 2], F32, tag="agg_in", name=f"agg{f}")
        nc.vector.tensor_copy(out=agg_in[:, :, 0], in_=mv[:, :, 0])
        nc.vector.tensor_tensor(
            out=agg_in[:, :, 1], in0=mv[:, :, 0], in1=mv[:, :, 0], op=OP.mult
        )
        nc.vector.tensor_tensor(
            out=agg_in[:, :, 1], in0=agg_in[:, :, 1], in1=mv[:, :, 1], op=OP.add
        )
        # group-aggregate 16 channels (partitions) per group: [8, (ko,stat)]
        gps = psg.tile([8, 8], F32, tag="gps", name=f"gps{f}")
        if f == 0:
            # warm-up: absorb each const/weight DMA wait into its own dummy
            # matmul so every real matmul carries at most one sync wait
            # (the fused-LDW matmul instruction has a single wait slot).
            warm_slices = [ws[wnm][:, 0, :8] for wnm in ("wq", "wk", "wv", "wp")]
            warm_slices.append(ones_s[:, 0, :8])
            for wsl in warm_slices:
                nc.tensor.matmul(
                    gps[:], lhsT=wsl, rhs=wsl, start=True, stop=True,
                )
            nc.tensor.matmul(
                gps[:], lhsT=aggA_s[:], rhs=aggA_s[:], start=True, stop=True,
            )
        nc.tensor.matmul(
            gps[:],
            lhsT=aggA_s[:],
            rhs=agg_in[:].rearrange("p a b -> p (a b)"),
            start=True,
            stop=True,
        )
        gpsv = gps[:].rearrange("p (a b) -> p a b", b=2)
        # var_g = E[x^2]_g - mean_g^2 (DVE may read only one PSUM operand)
        mean8 = pgn.tile([8, KO], F32, tag="mean8", name=f"mean8{f}")
        tmp8 = pgn.tile([8, KO], F32, tag="tmp8", name=f"tmp8{f}")
        vv = pgn.tile([8, KO], F32, tag="vv", name=f"vv{f}")
        nc.vector.tensor_copy(out=mean8[:], in_=gpsv[:, :, 0])
        nc.vector.tensor_tensor(
            out=tmp8[:], in0=mean8[:], in1=mean8[:], op=OP.mult
        )
        nc.vector.tensor_tensor(
            out=vv[:], in0=gpsv[:, :, 1], in1=tmp8[:], op=OP.subtract
        )
        # gs2 written only by ACT so the expand matmul needs a single wait:
        # col0 = mean (copy from PSUM), col1 = rstd = exp(-0.5*ln(var+eps))
        gs2 = pgn.tile([8, KO, 2], F32, tag="gs2", name=f"gs2{f}")
        nc.scalar.activation(out=gs2[:, :, 0], in_=gpsv[:, :, 0], func=AF.Copy)
        nc.scalar.activation(out=gs2[:, :, 1], in_=vv[:], func=AF.Ln, bias=eps8[:])
        nc.scalar.activation(
            out=gs2[:, :, 1], in_=gs2[:, :, 1], func=AF.Exp, scale=-0.5
        )
        # broadcast group stats back to the 128 channel partitions
        gpe = psg2.tile([P, 8], F32, tag="gpe", name=f"gpe{f}")
        if f == 0:
            nc.tensor.matmul(
                gpe[:8], lhsT=expB_s[:, :8], rhs=expB_s[:, :8],
                start=True, stop=True,
            )
        nc.tensor.matmul(
            gpe[:],
            lhsT=expB_s[:],
            rhs=gs2[:].rearrange("p a b -> p (a b)"),
            start=True,
            stop=True,
        )
        gpev = gpe[:].rearrange("p (a b) -> p a b", b=2)
        # fold GN affine: h = x*(rstd*s) + (b - mean*rstd*s)
        scp = pgn.tile([P, KO], F32, tag="scp", name=f"scp{f}")
        bip = pgn.tile([P, KO], F32, tag="bip", name=f"bip{f}")
        tmpp = pgn.tile([P, KO], F32, tag="tmpp", name=f"tmpp{f}")
        nc.vector.tensor_tensor(
            out=scp[:], in0=gpev[:, :, 1], in1=small["gns"][:], op=OP.mult
        )
        nc.vector.tensor_tensor(
            out=tmpp[:], in0=gpev[:, :, 0], in1=scp[:], op=OP.mult
        )
        nc.vector.tensor_tensor(
            out=bip[:], in0=small["gnb"][:], in1=tmpp[:], op=OP.subtract
        )
        hf = phf.tile([P, KO, HW], F8, tag="hfout", name=f"hf{f}")
        for ko in range(KO):
            nc.gpsimd.tensor_scalar(
                out=hf[:, ko, :],
                in0=xf[:, ko, :],
                scalar1=scp[:, ko : ko + 1],
                scalar2=bip[:, ko : ko + 1],
                op0=OP.mult,
                op1=OP.add,
            )
        xfs[f], hfs[f] = xf, hf

    def qkv(f):
        hf = hfs[f]
        qf = pq.tile([P, KO, HW], F8, tag="qf", name=f"qf{f}")
        kf = pk.tile([P, KO, HW], F8, tag="kf", name=f"kf{f}")
        for wt, bt, dst in ((ws["wq"], small["bq"], qf), (ws["wk"], small["bk"], kf)):
            for mi in range(KO):
                for ic in range(NCH):
                    pt = psum.tile([P, 512], F32, tag="pb", name="pt")
                    for ka in range(KO // 2):
                        nc.tensor.matmul(
                            pt[:],
                            lhsT=wt[:, 2 * ka : 2 * ka + 2, ts(mi, P)],
                            rhs=hf[:, 2 * ka : 2 * ka + 2, ts(ic, 512)],
                            start=(ka == 0),
                            stop=(ka == KO // 2 - 1),
                            perf_mode=DR,
                        )
                    nc.vector.tensor_scalar(
                        out=dst[:, mi, ts(ic, 512)],
                        in0=pt[:],
                        scalar1=1.0 / WS,
                        scalar2=bt[:, mi : mi + 1],
                        op0=OP.mult,
                        op1=OP.add,
                    )
        vt = pv.tile([P, SO, C], F8, tag="vt", name=f"vt{f}")
        for so in range(SO):
            pt = psum.tile([P, 512], F32, tag="pb", name="pt")
            for ka in range(KO // 2):
                nc.tensor.matmul(
                    pt[:],
                    lhsT=hf[:, 2 * ka : 2 * ka + 2, ts(so, P)],
                    rhs=ws["wv"][:, 2 * ka : 2 * ka + 2, :],
                    start=(ka == 0),
                    stop=(ka == KO // 2 - 1),
                    perf_mode=DR,
                )
            # vt = 64*(v + bv): the 64 cancels against Z (ones=2.0) below
            nc.vector.tensor_add(out=vt[:, so, :], in0=pt[:], in1=bvb_s[:])
        qs[f], ks[f], vts[f] = qf, kf, vt

    def attn(f):
        qf, kf, vt = qs[f], ks[f], vts[f]
        att = pat.tile([P, KO, HW], F8, tag="att", name=f"att{f}")
        for ic in range(NCH):
            # E = exp(scale * S^T) for this i-chunk, j on partitions
            ef = pe_.tile([P, SO, 512], F8, tag="ef", name=f"ef{f}_{ic}")
            for jo in range(SO):
                pt = psum.tile([P, 512], F32, tag="pb", name="pt")
                for ka in range(KO // 2):
                    nc.tensor.matmul(
                        pt[:],
                        lhsT=kf[:, 2 * ka : 2 * ka + 2, ts(jo, P)],
                        rhs=qf[:, 2 * ka : 2 * ka + 2, ts(ic, 512)],
                        start=(ka == 0),
                        stop=(ka == KO // 2 - 1),
                        perf_mode=DR,
                    )
                nc.scalar.activation(out=ef[:, jo, :], in_=pt[:], func=AF.Exp, scale=SCALE)
            # Z2_i = 2*sum_j E[j,i], broadcast to all partitions (ones=2.0)
            pz = psum.tile([P, 512], F32, tag="pb", name="pt")
            for ja in range(SO // 2):
                nc.tensor.matmul(
                    pz[:],
                    lhsT=ones_s[:],
                    rhs=ef[:, 2 * ja : 2 * ja + 2, :],
                    start=(ja == 0),
                    stop=(ja == SO // 2 - 1),
                    perf_mode=DR,
                )
            rz = prz.tile([P, 512], F32, tag="rz", name=f"rz{f}_{ic}")
            nc.vector.reciprocal_approx_fast(out=rz[:], in_=pz[:])
            # att = (sum_j vt[j,c] E[j,i]) * rz = 32 * att_true
            for mi in range(KO):
                pt = psum.tile([P, 512], F32, tag="pb", name="pt")
                for ja in range(SO // 2):
                    nc.tensor.matmul(
                        pt[:],
                        lhsT=vt[:, 2 * ja : 2 * ja + 2, ts(mi, P)],
                        rhs=ef[:, 2 * ja : 2 * ja + 2, :],
                        start=(ja == 0),
                        stop=(ja == SO // 2 - 1),
                        perf_mode=DR,
                    )
                nc.vector.tensor_mul(
                    out=att[:, mi, ts(ic, 512)], in0=pt[:], in1=rz[:]
                )
        atts[f] = att

    def proj(f):
        att, xf = atts[f], xfs[f]
        of = pof.tile([P, KO, HW], F32, tag="of", name=f"of{f}")
        for mi in range(KO):
            for ic in range(NCH):
                pt = psum.tile([P, 512], F32, tag="pb", name="pt")
                for ka in range(KO // 2):
                    nc.tensor.matmul(
                        pt[:],
                        lhsT=ws["wp"][:, 2 * ka : 2 * ka + 2, ts(mi, P)],
                        rhs=att[:, 2 * ka : 2 * ka + 2, ts(ic, 512)],
                        start=(ka == 0),
                        stop=(ka == KO // 2 - 1),
                        perf_mode=DR,
                    )
                nc.scalar.activation(
                    out=of[:, mi, ts(ic, 512)],
                    in_=pt[:],
                    func=AF.Identity,
                    bias=small["bp"][:, mi : mi + 1],
                    scale=1.0 / (WS * 32.0),
                )
                nc.gpsimd.tensor_tensor(
                    out=of[:, mi, ts(ic, 512)],
                    in0=of[:, mi, ts(ic, 512)],
                    in1=xf[:, mi, ts(ic, 512)],
                    op=OP.add,
                )
            nc.sync.dma_start(
                out=d["out"].ap()[f % FPC, :, mi], in_=of[:, mi]
            )

    # Emission order = scheduling priority. Hoist frame f+1's load+GN ahead of
    # frame f's attention so the frame-boundary normalize overlaps PE work.
    nvf = FPC * reps
    load_gn(0)
    qkv(0)
    if nvf > 1:
        load_gn(1)
    for f in range(nvf):
        attn(f)
        proj(f)
        if f + 1 < nvf:
            qkv(f + 1)
        if f + 2 < nvf:
            load_gn(f + 2)


_NC_CACHE = None


def _get_nc():
    global _NC_CACHE
    if _NC_CACHE is None:
        _NC_CACHE = _build()
    return _NC_CACHE


def _f8(a):
    return np.clip(np.asarray(a, np.float32), -240.0, 240.0).astype(
        ml_dtypes.float8_e4m3
    )


def _wprep(w):
    # w [Cout, Cin] -> lhsT layout [P, KO(ki), Cout], cin = ki*128 + p
    w = np.asarray(w, np.float32) * WS
    return _f8(np.ascontiguousarray(w.T.reshape(KO, P, C).transpose(1, 0, 2)))


def _bprep(b):
    # b [C] -> [P, KO], c = ko*128 + p
    return np.ascontiguousarray(np.asarray(b, np.float32).reshape(KO, P).T)


def _prep(inputs):
    x = np.asarray(inputs["x"], dtype=np.float32)
    base = {
        "wq": _wprep(inputs["wq"]),
        "wk": _wprep(inputs["wk"]),
        "wv": _wprep(inputs["wv"]),
        "wp": _wprep(inputs["wproj"]),
        "bq": _bprep(inputs["bq"]),
        "bk": _bprep(inputs["bk"]),
        "bp": _bprep(inputs["bproj"]),
        "gns": _bprep(inputs["gn_scale"]),
        "gnb": _bprep(inputs["gn_bias"]),
        "ones": _f8(np.full((P, 2, P), 2.0, np.float32)),
        "bvb": np.ascontiguousarray(
            np.broadcast_to(
                np.asarray(inputs["bv"], np.float32) * WS, (P, C)
            )
        ),
    }
    xs = x.reshape(FRAMES, KO, P, HW).transpose(0, 2, 1, 3)  # [16, P, KO, HW]
    in_maps = []
    for i in range(NCORES):
        m = dict(base)
        m["x"] = np.ascontiguousarray(xs[i * FPC : (i + 1) * FPC])
        in_maps.append(m)
    return in_maps


def _run(inputs, trace=False):
    nc = _get_nc()
    in_maps = _prep(inputs)
    res = run_bass_kernel_spmd(
        nc, in_maps, core_ids=list(range(NCORES)), trace=trace
    )
    outs = []
    for rmap in res.results:
        o = np.asarray(rmap["out"])  # [FPC, P, KO, HW]
        outs.append(o.transpose(0, 2, 1, 3).reshape(FPC, C, H, W))
    full = np.concatenate(outs, axis=0).reshape(B, T, C, H, W).astype(np.float32)
    return full, res


def kernel(**inputs):
    out, _ = _run(inputs, trace=False)
    return out


# revision 10
# speedup vs baseline: 1.4242x; 1.0195x over previous
"""AttnBlock (GroupNorm + 1x1-conv QKV + spatial attention + proj + residual)
as a Bass/Tile kernel for 8 Trainium2 NeuronCores.

Sharding: data-parallel over the folded B*T=16 frame axis -> 2 frames/core.
Params replicated. Each core runs an identical program on its own frame pair.

All heavy matmuls run in fp8e4 (e4m3) DoubleRow mode: the PE array is
virtualized to 128x256 (2 fp8 weights per cell), halving streaming cycles.
Weights are host-prescaled by 64 to keep them out of the fp8 subnormal range;
the scale is removed in the drains (q/k: *1/64, proj: *1/2048). V keeps the
64x (drain adds 64*bv), which cancels in softmax normalization because the
row-sum Z is accumulated with a ones=2.0 matrix: att = (64*AV)*(1/(2Z)) =
32*att_true, and the proj drain divides by 64*32 = 2048.

Layout conventions (per frame):
  x                    : SBUF fp32 [128, KO=4, HW=1024], channel c = ko*128+p
  h, q, k, att         : SBUF fp8  [128, KO=4, HW=1024]
  vT                   : SBUF fp8  [128, SO=8, C=512],   spatial s = so*128+p
  E = exp(scale*S^T)   : SBUF fp8  [128, SO=8, 512],     j = jo*128+p
Softmax has no max-subtraction (scores are O(1) for this problem).

Engine split: PE matmuls; ACT exp + proj drain; DVE q/k drains, v drain,
AV drain, bn_stats, reciprocal_approx_fast; GpSimd GN-normalize + residual.
"""

from contextlib import ExitStack

import numpy as np
import ml_dtypes

import concourse.bass as bass
import concourse.bacc as bacc
import concourse.mybir as mybir
import concourse.tile as tile
from concourse.bass import ts
from concourse.bass_utils import run_bass_kernel_spmd

# Problem shapes (hardcoded per harness contract)
B, T, C, H, W = 2, 8, 512, 32, 32
HW = H * W              # 1024
FRAMES = B * T          # 16
NCORES = 8
FPC = FRAMES // NCORES  # frames per core
P = 128
KO = C // P             # 4 channel blocks
SO = HW // P            # 8 spatial blocks
NCH = HW // 512         # 2 free chunks of 512
EPS = 1e-6
SCALE = float(C) ** -0.5
WS = 64.0               # host-side weight prescale (keeps fp8 out of denormals)

F32 = mybir.dt.float32
F8 = mybir.dt.float8e4
AF = mybir.ActivationFunctionType
OP = mybir.AluOpType
DR = mybir.MatmulPerfMode.DoubleRow


def _build(reps=1):
    nc = bacc.Bacc(None, target_bir_lowering=False)
    d = {}
    d["x"] = nc.dram_tensor("x", [FPC, P, KO, HW], F32, kind="ExternalInput")
    for nm in ("wq", "wk", "wv", "wp"):
        d[nm] = nc.dram_tensor(nm, [P, KO, C], F8, kind="ExternalInput")
    for nm in ("bq", "bk", "bp", "gns", "gnb"):
        d[nm] = nc.dram_tensor(nm, [P, KO], F32, kind="ExternalInput")
    d["ones"] = nc.dram_tensor("ones", [P, 2, P], F8, kind="ExternalInput")
    d["out"] = nc.dram_tensor("out", [FPC, P, KO, HW], F32, kind="ExternalOutput")

    # Constant matrices for the group-stat partition reductions (baked in NEFF)
    aggA = np.zeros((P, 8), np.float32)
    for pp in range(P):
        aggA[pp, pp // 16] = 1.0 / 16.0
    expB = np.zeros((8, P), np.float32)
    for pp in range(P):
        expB[pp // 16, pp] = 1.0
    d["aggA"] = nc.inline_tensor(aggA, "aggA")
    d["expB"] = nc.inline_tensor(expB, "expB")

    with tile.TileContext(nc) as tc:
        with ExitStack() as ctx:
            _emit(ctx, nc, tc, d, reps)
    nc.compile()
    return nc


def _emit(ctx, nc, tc, d, reps=1):
    const = ctx.enter_context(tc.tile_pool(name="const", bufs=1))
    px = ctx.enter_context(tc.tile_pool(name="px", bufs=3))
    phf = ctx.enter_context(tc.tile_pool(name="phf", bufs=2))
    pq = ctx.enter_context(tc.tile_pool(name="pqp", bufs=2))
    pk = ctx.enter_context(tc.tile_pool(name="pkp", bufs=2))
    pv = ctx.enter_context(tc.tile_pool(name="pvp", bufs=2))
    pe_ = ctx.enter_context(tc.tile_pool(name="pep", bufs=2))
    pat = ctx.enter_context(tc.tile_pool(name="patp", bufs=2))
    prz = ctx.enter_context(tc.tile_pool(name="przp", bufs=2))
    pgn = ctx.enter_context(tc.tile_pool(name="pgn", bufs=2))
    pof = ctx.enter_context(tc.tile_pool(name="pofp", bufs=2))
    psum = ctx.enter_context(tc.tile_pool(name="psum", bufs=5, space="PSUM"))
    psg = ctx.enter_context(tc.tile_pool(name="psg", bufs=1, space="PSUM"))
    psg2 = ctx.enter_context(tc.tile_pool(name="psg2", bufs=2, space="PSUM"))

    import math as _math

    # ---- DMA issue order matters: per-ring FIFO. ----
    # Frame-0 x slices are the startup critical path (GN stats gate QKV):
    # spread them FIRST across all three DMA-capable rings so they land in
    # parallel; weights follow on the scalar/gpsimd rings.
    xfs = {}
    x0 = px.tile([P, KO, HW], F32, tag="xf", name="xf0")
    nc.sync.dma_start(out=x0[:, 0], in_=d["x"].ap()[0, :, 0])
    nc.scalar.dma_start(out=x0[:, 2], in_=d["x"].ap()[0, :, 2])
    nc.gpsimd.dma_start(out=x0[:, 3], in_=d["x"].ap()[0, :, 3])
    nc.sync.dma_start(out=x0[:, 1], in_=d["x"].ap()[0, :, 1])
    xfs[0] = x0
    if FPC * reps > 1:
        xf = px.tile([P, KO, HW], F32, tag="xf", name="xf1")
        for ko in range(KO):
            nc.sync.dma_start(out=xf[:, ko], in_=d["x"].ap()[1 % FPC, :, ko])
        xfs[1] = xf

    ws = {}
    for nm, eng in (("wq", nc.scalar), ("wk", nc.scalar),
                    ("wv", nc.gpsimd), ("wp", nc.gpsimd)):
        t = const.tile([P, KO, C], F8, tag=f"{nm}_s", name=f"{nm}_s")
        eng.dma_start(out=t[:], in_=d[nm].ap())
        ws[nm] = t
    ones_s = const.tile([P, 2, P], F8, tag="ones_s", name="ones_s")
    nc.gpsimd.dma_start(out=ones_s[:], in_=d["ones"].ap())
    aggA_s = const.tile([P, 8], F32, tag="aggA_s", name="aggA_s")
    nc.scalar.dma_start(out=aggA_s[:], in_=d["aggA"].ap())
    expB_s = const.tile([8, P], F32, tag="expB_s", name="expB_s")
    nc.scalar.dma_start(out=expB_s[:], in_=d["expB"].ap())
    small = {}
    for nm in ("bq", "bk", "bp", "gns", "gnb"):
        t = const.tile([P, KO], F32, tag=f"{nm}_s", name=f"{nm}_s")
        nc.scalar.dma_start(out=t[:], in_=d[nm].ap())
        small[nm] = t

    # Warm the ACT table set (Exp/Ln/Identity/Copy) during the DMA window:
    # eps8 = exp(ln(eps)); tiny Identity+Copy on the result.
    epsl = const.tile([8, 1], F32, tag="epsl", name="epsl")
    eps8 = const.tile([8, 1], F32, tag="eps8", name="eps8")
    scr8 = const.tile([8, 1], F32, tag="scr8", name="scr8")
    nc.vector.memset(epsl[:], _math.log(EPS))
    nc.scalar.activation(out=eps8[:], in_=epsl[:], func=AF.Exp)
    nc.scalar.activation(out=scr8[:], in_=eps8[:], func=AF.Identity)
    nc.scalar.activation(out=scr8[:], in_=eps8[:], func=AF.Copy)

    hfs, qs, ks, vts, atts = {}, {}, {}, {}, {}

    def load_gn(f):
        """DMA frame f in; GroupNorm stats + normalize (fused affine)."""
        if f in xfs:
            xf = xfs[f]
        else:
            xf = px.tile([P, KO, HW], F32, tag="xf", name=f"xf{f}")
            for ko in range(KO):
                nc.sync.dma_start(out=xf[:, ko], in_=d["x"].ap()[f % FPC, :, ko])
        # per-channel mean/var over the 1024 spatial positions
        stats = pgn.tile([P, KO, 2, 6], F32, tag="stats", name=f"stats{f}")
        for ko in range(KO):
            for hh in range(2):
                nc.vector.bn_stats(out=stats[:, ko, hh, :], in_=xf[:, ko, ts(hh, 512)])
        mv = pgn.tile([P, KO, 2], F32, tag="mv", name=f"mv{f}")
        for ko in range(KO):
            nc.vector.bn_aggr(out=mv[:, ko, :], in_=stats[:, ko, :, :])
        # columns: (mean_c, var_c + mean_c^2) = (mean_c, E[x^2]_c)
        agg_in = pgn.tile([P, KO, 2], F32, tag="agg_in", name=f"agg{f}")
        nc.vector.tensor_copy(out=agg_in[:, :, 0], in_=mv[:, :, 0])
        nc.vector.tensor_tensor(
            out=agg_in[:, :, 1], in0=mv[:, :, 0], in1=mv[:, :, 0], op=OP.mult
        )
        nc.vector.tensor_tensor(
            out=agg_in[:, :, 1], in0=agg_in[:, :, 1], in1=mv[:, :, 1], op=OP.add
        )
        # group-aggregate 16 channels (partitions) per group: [8, (ko,stat)]
        gps = psg.tile([8, 8], F32, tag="gps", name=f"gps{f}")
        if f == 0:
            # warm-up: absorb each const/weight DMA wait into its own dummy
            # matmul so every real matmul carries at most one sync wait
            # (the fused-LDW matmul instruction has a single wait slot).
            warm_slices = [ws[wnm][:, 0, :8] for wnm in ("wq", "wk", "wv", "wp")]
            warm_slices.append(ones_s[:, 0, :8])
            for wsl in warm_slices:
                nc.tensor.matmul(
                    gps[:], lhsT=wsl, rhs=wsl, start=True, stop=True,
                )
            nc.tensor.matmul(
                gps[:], lhsT=aggA_s[:], rhs=aggA_s[:], start=True, stop=True,
            )
        nc.tensor.matmul(
            gps[:],
            lhsT=aggA_s[:],
            rhs=agg_in[:].rearrange("p a b -> p (a b)"),
            start=True,
            stop=True,
        )
        gpsv = gps[:].rearrange("p (a b) -> p a b", b=2)
        # var_g = E[x^2]_g - mean_g^2 (DVE may read only one PSUM operand)
        mean8 = pgn.tile([8, KO], F32, tag="mean8", name=f"mean8{f}")
        tmp8 = pgn.tile([8, KO], F32, tag="tmp8", name=f"tmp8{f}")
        vv = pgn.tile([8, KO], F32, tag="vv", name=f"vv{f}")
        nc.vector.tensor_copy(out=mean8[:], in_=gpsv[:, :, 0])
        nc.vector.tensor_tensor(
            out=tmp8[:], in0=mean8[:], in1=mean8[:], op=OP.mult
        )
        nc.vector.tensor_tensor(
            out=vv[:], in0=gpsv[:, :, 1], in1=tmp8[:], op=OP.subtract
        )
        # gs2 written only by ACT so the expand matmul needs a single wait:
        # col0 = mean (copy from PSUM), col1 = rstd = exp(-0.5*ln(var+eps))
        gs2 = pgn.tile([8, KO, 2], F32, tag="gs2", name=f"gs2{f}")
        nc.scalar.activation(out=gs2[:, :, 0], in_=gpsv[:, :, 0], func=AF.Copy)
        nc.scalar.activation(out=gs2[:, :, 1], in_=vv[:], func=AF.Ln, bias=eps8[:])
        nc.scalar.activation(
            out=gs2[:, :, 1], in_=gs2[:, :, 1], func=AF.Exp, scale=-0.5
        )
        # broadcast group stats back to the 128 channel partitions
        gpe = psg2.tile([P, 8], F32, tag="gpe", name=f"gpe{f}")
        if f == 0:
            nc.tensor.matmul(
                gpe[:8], lhsT=expB_s[:, :8], rhs=expB_s[:, :8],
                start=True, stop=True,
            )
        nc.tensor.matmul(
            gpe[:],
            lhsT=expB_s[:],
            rhs=gs2[:].rearrange("p a b -> p (a b)"),
            start=True,
            stop=True,
        )
        gpev = gpe[:].rearrange("p (a b) -> p a b", b=2)
        # fold GN affine: h = x*(rstd*s) + (b - mean*rstd*s)
        scp = pgn.tile([P, KO], F32, tag="scp", name=f"scp{f}")
        bip = pgn.tile([P, KO], F32, tag="bip", name=f"bip{f}")
        tmpp = pgn.tile([P, KO], F32, tag="tmpp", name=f"tmpp{f}")
        nc.vector.tensor_tensor(
            out=scp[:], in0=gpev[:, :, 1], in1=small["gns"][:], op=OP.mult
        )
        nc.vector.tensor_tensor(
            out=tmpp[:], in0=gpev[:, :, 0], in1=scp[:], op=OP.mult
        )
        nc.vector.tensor_tensor(
            out=bip[:], in0=small["gnb"][:], in1=tmpp[:], op=OP.subtract
        )
        hf = phf.tile([P, KO, HW], F8, tag="hfout", name=f"hf{f}")
        for ko in range(KO):
            nc.gpsimd.tensor_scalar(
                out=hf[:, ko, :],
                in0=xf[:, ko, :],
                scalar1=scp[:, ko : ko + 1],
                scalar2=bip[:, ko : ko + 1],
                op0=OP.mult,
                op1=OP.add,
            )
        xfs[f], hfs[f] = xf, hf

    def qkv(f):
        # q/k are stored as 64*(q+bq) in fp8 (|.|<~170<240); the 4096x on the
        # scores is removed in the exp drain's scale. v stays 64*v; bv is
        # folded into the proj bias host-side (softmax weights sum to 1).
        hf = hfs[f]
        qf = pq.tile([P, KO, HW], F8, tag="qf", name=f"qf{f}")
        kf = pk.tile([P, KO, HW], F8, tag="kf", name=f"kf{f}")
        for mi in range(KO):
            for ic in range(NCH):
                pt = psum.tile([P, 512], F32, tag="pb", name="pt")
                for ka in range(KO // 2):
                    nc.tensor.matmul(
                        pt[:],
                        lhsT=ws["wq"][:, 2 * ka : 2 * ka + 2, ts(mi, P)],
                        rhs=hf[:, 2 * ka : 2 * ka + 2, ts(ic, 512)],
                        start=(ka == 0),
                        stop=(ka == KO // 2 - 1),
                        perf_mode=DR,
                    )
                nc.vector.tensor_scalar_add(
                    out=qf[:, mi, ts(ic, 512)],
                    in0=pt[:],
                    scalar1=small["bq"][:, mi : mi + 1],
                )
        for mi in range(KO):
            for ic in range(NCH):
                pt = psum.tile([P, 512], F32, tag="pb", name="pt")
                for ka in range(KO // 2):
                    nc.tensor.matmul(
                        pt[:],
                        lhsT=ws["wk"][:, 2 * ka : 2 * ka + 2, ts(mi, P)],
                        rhs=hf[:, 2 * ka : 2 * ka + 2, ts(ic, 512)],
                        start=(ka == 0),
                        stop=(ka == KO // 2 - 1),
                        perf_mode=DR,
                    )
                nc.scalar.activation(
                    out=kf[:, mi, ts(ic, 512)],
                    in_=pt[:],
                    func=AF.Identity,
                    bias=small["bk"][:, mi : mi + 1],
                )
        vt = pv.tile([P, SO, C], F8, tag="vt", name=f"vt{f}")
        for so in range(SO):
            pt = psum.tile([P, 512], F32, tag="pb", name="pt")
            for ka in range(KO // 2):
                nc.tensor.matmul(
                    pt[:],
                    lhsT=hf[:, 2 * ka : 2 * ka + 2, ts(so, P)],
                    rhs=ws["wv"][:, 2 * ka : 2 * ka + 2, :],
                    start=(ka == 0),
                    stop=(ka == KO // 2 - 1),
                    perf_mode=DR,
                )
            nc.scalar.activation(out=vt[:, so, :], in_=pt[:], func=AF.Copy)
        qs[f], ks[f], vts[f] = qf, kf, vt

    def attn(f):
        qf, kf, vt = qs[f], ks[f], vts[f]
        att = pat.tile([P, KO, HW], F8, tag="att", name=f"att{f}")
        for ic in range(NCH):
            # E = exp(scale * S^T) for this i-chunk, j on partitions
            ef = pe_.tile([P, SO, 512], F8, tag="ef", name=f"ef{f}_{ic}")
            for jo in range(SO):
                pt = psum.tile([P, 512], F32, tag="pb", name="pt")
                for ka in range(KO // 2):
                    nc.tensor.matmul(
                        pt[:],
                        lhsT=kf[:, 2 * ka : 2 * ka + 2, ts(jo, P)],
                        rhs=qf[:, 2 * ka : 2 * ka + 2, ts(ic, 512)],
                        start=(ka == 0),
                        stop=(ka == KO // 2 - 1),
                        perf_mode=DR,
                    )
                nc.scalar.activation(
                    out=ef[:, jo, :], in_=pt[:], func=AF.Exp,
                    scale=SCALE / (WS * WS),
                )
            # Z2_i = 2*sum_j E[j,i], broadcast to all partitions (ones=2.0)
            pz = psum.tile([P, 512], F32, tag="pb", name="pt")
            for ja in range(SO // 2):
                nc.tensor.matmul(
                    pz[:],
                    lhsT=ones_s[:],
                    rhs=ef[:, 2 * ja : 2 * ja + 2, :],
                    start=(ja == 0),
                    stop=(ja == SO // 2 - 1),
                    perf_mode=DR,
                )
            rz = prz.tile([P, 512], F32, tag="rz", name=f"rz{f}_{ic}")
            nc.vector.reciprocal_approx_fast(out=rz[:], in_=pz[:])
            # att = (sum_j vt[j,c] E[j,i]) * rz = 32 * att_true
            for mi in range(KO):
                pt = psum.tile([P, 512], F32, tag="pb", name="pt")
                for ja in range(SO // 2):
                    nc.tensor.matmul(
                        pt[:],
                        lhsT=vt[:, 2 * ja : 2 * ja + 2, ts(mi, P)],
                        rhs=ef[:, 2 * ja : 2 * ja + 2, :],
                        start=(ja == 0),
                        stop=(ja == SO // 2 - 1),
                        perf_mode=DR,
                    )
                nc.vector.tensor_mul(
                    out=att[:, mi, ts(ic, 512)], in0=pt[:], in1=rz[:]
                )
        atts[f] = att

    def proj(f):
        att, xf = atts[f], xfs[f]
        of = pof.tile([P, KO, HW], F32, tag="of", name=f"of{f}")
        for mi in range(KO):
            for ic in range(NCH):
                pt = psum.tile([P, 512], F32, tag="pb", name="pt")
                for ka in range(KO // 2):
                    nc.tensor.matmul(
                        pt[:],
                        lhsT=ws["wp"][:, 2 * ka : 2 * ka + 2, ts(mi, P)],
                        rhs=att[:, 2 * ka : 2 * ka + 2, ts(ic, 512)],
                        start=(ka == 0),
                        stop=(ka == KO // 2 - 1),
                        perf_mode=DR,
                    )
                nc.scalar.activation(
                    out=of[:, mi, ts(ic, 512)],
                    in_=pt[:],
                    func=AF.Identity,
                    bias=small["bp"][:, mi : mi + 1],
                    scale=1.0 / (WS * 32.0),
                )
                # residual add; alternate engines so the last frame's chain
                # isn't serialized on one engine
                reng = nc.gpsimd if (mi + ic) % 2 == 0 else nc.vector
                reng.tensor_tensor(
                    out=of[:, mi, ts(ic, 512)],
                    in0=of[:, mi, ts(ic, 512)],
                    in1=xf[:, mi, ts(ic, 512)],
                    op=OP.add,
                )
                nc.sync.dma_start(
                    out=d["out"].ap()[f % FPC, :, mi, ts(ic, 512)],
                    in_=of[:, mi, ts(ic, 512)],
                )

    # Emission order = scheduling priority. Hoist frame f+1's load+GN ahead of
    # frame f's attention so the frame-boundary normalize overlaps PE work.
    nvf = FPC * reps
    load_gn(0)
    qkv(0)
    if nvf > 1:
        load_gn(1)
    for f in range(nvf):
        attn(f)
        proj(f)
        if f + 1 < nvf:
            qkv(f + 1)
        if f + 2 < nvf:
            load_gn(f + 2)


_NC_CACHE = None


def _get_nc():
    global _NC_CACHE
    if _NC_CACHE is None:
        _NC_CACHE = _build()
    return _NC_CACHE


def _f8(a):
    return np.clip(np.asarray(a, np.float32), -240.0, 240.0).astype(
        ml_dtypes.float8_e4m3
    )


def _wprep(w):
    # w [Cout, Cin] -> lhsT layout [P, KO(ki), Cout], cin = ki*128 + p
    w = np.asarray(w, np.float32) * WS
    return _f8(np.ascontiguousarray(w.T.reshape(KO, P, C).transpose(1, 0, 2)))


def _bprep(b):
    # b [C] -> [P, KO], c = ko*128 + p
    return np.ascontiguousarray(np.asarray(b, np.float32).reshape(KO, P).T)


def _prep(inputs):
    x = np.asarray(inputs["x"], dtype=np.float32)
    # fold bv through the attention output (softmax rows sum to 1):
    # proj(att + bv) = proj(att) + wproj @ bv
    bp_eff = np.asarray(inputs["bproj"], np.float32) + (
        np.asarray(inputs["wproj"], np.float32)
        @ np.asarray(inputs["bv"], np.float32)
    )
    base = {
        "wq": _wprep(inputs["wq"]),
        "wk": _wprep(inputs["wk"]),
        "wv": _wprep(inputs["wv"]),
        "wp": _wprep(inputs["wproj"]),
        "bq": _bprep(np.asarray(inputs["bq"], np.float32) * WS),
        "bk": _bprep(np.asarray(inputs["bk"], np.float32) * WS),
        "bp": _bprep(bp_eff),
        "gns": _bprep(inputs["gn_scale"]),
        "gnb": _bprep(inputs["gn_bias"]),
        "ones": _f8(np.full((P, 2, P), 2.0, np.float32)),
    }
    xs = x.reshape(FRAMES, KO, P, HW).transpose(0, 2, 1, 3)  # [16, P, KO, HW]
    in_maps = []
    for i in range(NCORES):
        m = dict(base)
        m["x"] = np.ascontiguousarray(xs[i * FPC : (i + 1) * FPC])
        in_maps.append(m)
    return in_maps


def _run(inputs, trace=False):
    nc = _get_nc()
    in_maps = _prep(inputs)
    res = run_bass_kernel_spmd(
        nc, in_maps, core_ids=list(range(NCORES)), trace=trace
    )
    outs = []
    for rmap in res.results:
        o = np.asarray(rmap["out"])  # [FPC, P, KO, HW]
        outs.append(o.transpose(0, 2, 1, 3).reshape(FPC, C, H, W))
    full = np.concatenate(outs, axis=0).reshape(B, T, C, H, W).astype(np.float32)
    return full, res


def kernel(**inputs):
    out, _ = _run(inputs, trace=False)
    return out
